# revision 1
# baseline (speedup 1.0000x reference)
"""Trainium2 Bass kernel for fused MultiHeadAttention + residual + LayerNorm.

Problem: B=2, L=S=2048, D=768, H=12 heads of dim 64, attention with key-padding
mask, output projection + bias, residual with q, LayerNorm(gamma, beta).

Sharding over 8 NeuronCores: data-parallel over batch (2 groups of 4 cores) x
tensor-parallel over heads (3 heads per core). Each core:
  1. projects its 3 heads' Q^T/K^T (feature-major) and V (seq-major) with f32r
     matmuls,
  2. computes S^T = K^T.T @ Q^T per head (heads 0/1 PE-row-group paired),
     exp via ScalarE with the key-padding mask folded into the activation bias,
     and O^T = [V|1].T @ P^T accumulated over s-chunks (the extra ones column
     yields the softmax denominator for free),
  3. normalizes O^T rows by the broadcast reciprocal denominator, applies the
     output projection for its 192 feature columns,
  4. ReduceScatters the partial projection over its 4-core batch group, then
     does bias + residual + LayerNorm on its 512-row shard.
Host reassembles the 8 x [512, 768] shards into (2, 2048, 768).
"""

import sys

sys.path.insert(0, "/opt/trn_rl_repo")

import ml_dtypes
import numpy as np

import concourse.bass as bass
import concourse.tile as tile
from concourse import bacc, mybir
from concourse.bass_utils import run_bass_kernel_spmd

F32 = mybir.dt.float32
F32R = mybir.dt.float32r
BF16 = mybir.dt.bfloat16
I32 = mybir.dt.int32

D = 768
HD = 64
HPC = 3  # heads per core
HCOLS = HPC * HD  # 192
B, L, S = 2, 2048, 2048
ROWS = 512  # output rows per core after ReduceScatter
NCORES = 8
GROUPS = [[0, 1, 2, 3], [4, 5, 6, 7]]
KCH = D // 128  # 6 contraction chunks for projections
SCH = S // 128  # 16 s-chunks
LBW = 512  # l-block width
LBN = L // LBW  # 4
LN_EPS = 1e-5
MASK_NEG = -1000000.0

_CACHE: dict = {}


def _build():
    nc = bacc.Bacc("TRN2", target_bir_lowering=False, debug=False, num_devices=NCORES)

    qT = nc.dram_tensor("qT", [D, L], F32R, kind="ExternalInput").ap()
    kT = nc.dram_tensor("kT", [D, S], F32R, kind="ExternalInput").ap()
    vT = nc.dram_tensor("vT", [D, S], F32R, kind="ExternalInput").ap()
    wqT = nc.dram_tensor("wqT", [D, HCOLS], F32R, kind="ExternalInput").ap()
    wkT = nc.dram_tensor("wkT", [D, HCOLS], F32R, kind="ExternalInput").ap()
    wvT = nc.dram_tensor("wvT", [D, 256], F32R, kind="ExternalInput").ap()
    wTh = [
        nc.dram_tensor(f"wTh{h}", [128, D], F32R, kind="ExternalInput").ap()
        for h in range(HPC)
    ]
    qrows = nc.dram_tensor("qrows", [LBN, 128, D], F32, kind="ExternalInput").ap()
    maskT = nc.dram_tensor("maskT", [128, SCH], I32, kind="ExternalInput").ap()
    bias1 = nc.dram_tensor("bias1", [1, D], F32, kind="ExternalInput").ap()
    gamma1 = nc.dram_tensor("gamma1", [1, D], F32, kind="ExternalInput").ap()
    beta1 = nc.dram_tensor("beta1", [1, D], F32, kind="ExternalInput").ap()
    out = nc.dram_tensor("out", [LBN, 128, D], F32, kind="ExternalOutput").ap()

    AL = mybir.AluOpType
    ACT = mybir.ActivationFunctionType

    with tile.TileContext(nc, num_cores=NCORES) as tc:
        with (
            tc.tile_pool(name="persist", bufs=1) as pp,
            tc.tile_pool(name="dram", bufs=1, space="DRAM") as dram,
        ):
            # persistent SBUF state; head-C rows are duplicated to partitions
            # 64:128 of QT2/KT2 so its S^T matmuls can PE-row-group pair.
            QT1 = pp.tile([128, L], F32R)
            QT2 = pp.tile([128, L], F32R)
            KT1 = pp.tile([128, S], F32R)
            KT2 = pp.tile([128, S], F32R)
            V_sb = pp.tile([128, SCH, HPC, 65], F32R)
            OTn = [pp.tile([128, L], F32R, name=f"OTn{h}") for h in range(HPC)]
            wq_sb = pp.tile([128, KCH, HCOLS], F32R)
            wk_sb = pp.tile([128, KCH, HCOLS], F32R)
            wv_sb = pp.tile([128, KCH, 256], F32R)
            wTh_sb = [pp.tile([128, D], F32R, name=f"wTh_sb{h}") for h in range(HPC)]
            mask_i = pp.tile([128, SCH], I32)
            mask_f = pp.tile([128, SCH], F32)
            mask_bias = pp.tile([128, SCH], F32)
            gam_b = pp.tile([128, D], F32)
            bet_b = pp.tile([128, D], F32)
            bb_b = pp.tile([128, D], F32)
            eps_t = pp.tile([128, 1], F32)

            Z_dram = dram.tile([L, D], F32)
            Zr_dram = dram.tile([LBN, 128, D], F32)

            # constant / weight loads
            nc.sync.dma_start(out=wq_sb, in_=wqT.rearrange("(c p) m -> p c m", p=128))
            nc.sync.dma_start(out=wk_sb, in_=wkT.rearrange("(c p) m -> p c m", p=128))
            nc.sync.dma_start(out=wv_sb, in_=wvT.rearrange("(c p) m -> p c m", p=128))
            for h in range(HPC):
                nc.sync.dma_start(out=wTh_sb[h], in_=wTh[h][:, :])
            nc.sync.dma_start(out=mask_i, in_=maskT[:, :])
            nc.sync.dma_start(out=gam_b, in_=gamma1.to_broadcast([128, D]))
            nc.sync.dma_start(out=bet_b, in_=beta1.to_broadcast([128, D]))
            nc.sync.dma_start(out=bb_b, in_=bias1.to_broadcast([128, D]))
            nc.vector.memset(eps_t, LN_EPS)
            ones_t = pp.tile([128, SCH, HPC, 1], F32)
            nc.vector.memset(ones_t, 1.0)
            nc.vector.tensor_copy(V_sb[:, :, :, 64:65], ones_t)
            nc.vector.tensor_copy(mask_f, mask_i)
            # (1 - m) * MASK_NEG == m * (-MASK_NEG) + MASK_NEG
            nc.scalar.activation(
                mask_bias, mask_f, ACT.Copy, bias=float(MASK_NEG), scale=-MASK_NEG
            )

            # PE warm-up: ~40 dependency-free matmuls run during the initial
            # DMA window and push the PE HAM clock-gate to 2.4 GHz before the
            # real work (f32r streams at 2 cyc/row on a cold PE, 1 warm).
            warm_f = pp.tile([128, 512], F32)
            nc.vector.memset(warm_f, 0.0)
            warm_l = pp.tile([128, 128], F32R)
            warm_r = pp.tile([128, 512], F32R)
            nc.vector.tensor_copy(warm_l, warm_f[:, 0:128])
            nc.vector.tensor_copy(warm_r, warm_f)
            with tc.tile_pool(name="warmps", bufs=1, space="PSUM") as wps:
                for w in range(40):
                    wp = wps.tile([128, 512], F32, tag="w", bufs=2, name=f"w{w}")
                    nc.tensor.matmul(wp, warm_l, warm_r, start=True, stop=True)

            # ---- Stage A: projections ----
            for xin, wsb, d1, d2 in ((qT, wq_sb, QT1, QT2), (kT, wk_sb, KT1, KT2)):
                with (
                    tc.tile_pool(name="pin", bufs=1) as pin,
                    tc.tile_pool(name="psp", bufs=1, space="PSUM") as psp,
                ):
                    chunks = []
                    for i in range(KCH):
                        ch = pin.tile([128, L], F32R, tag="in", bufs=KCH, name=f"ch{i}")
                        nc.sync.dma_start(out=ch, in_=xin[128 * i : 128 * (i + 1), :])
                        chunks.append(ch)
                    for m, mp in ((0, 128), (1, 64)):
                        for n in range(LBN):
                            ps = psp.tile([128, 512], F32, tag="ps", bufs=3, name="ps")
                            nsl = slice(512 * n, 512 * (n + 1))
                            for i in range(KCH):
                                nc.tensor.matmul(
                                    ps[:mp],
                                    wsb[:, i, 128 * m : 128 * m + mp],
                                    chunks[i][:, nsl],
                                    start=(i == 0),
                                    stop=(i == KCH - 1),
                                )
                            dest = d1 if m == 0 else d2
                            nc.any.tensor_copy(out=dest[:mp, nsl], in_=ps[:mp])
                            if m == 1:
                                # duplicate head-C rows into partitions 64:128
                                nc.sync.dma_start(
                                    out=dest[64:128, nsl], in_=dest[0:64, nsl]
                                )

            with (
                tc.tile_pool(name="pinv", bufs=1) as pin,
                tc.tile_pool(name="pspv", bufs=1, space="PSUM") as psp,
            ):
                chunks = []
                for i in range(KCH):
                    ch = pin.tile([128, S], F32R, tag="in", bufs=KCH, name=f"vch{i}")
                    nc.sync.dma_start(out=ch, in_=vT[128 * i : 128 * (i + 1), :])
                    chunks.append(ch)
                for s in range(SCH):
                    ps = psp.tile([128, 256], F32, tag="ps", bufs=3, name="psv")
                    for i in range(KCH):
                        nc.tensor.matmul(
                            ps,
                            chunks[i][:, 128 * s : 128 * (s + 1)],
                            wv_sb[:, i, :],
                            start=(i == 0),
                            stop=(i == KCH - 1),
                        )
                    nc.any.tensor_copy(
                        out=V_sb[:, s, :, 0:64],
                        in_=ps[:, 0:HCOLS].rearrange("p (h d) -> p h d", h=HPC),
                    )
                # dense keep-warm bridge: spans the pool-transition idle gap so
                # the PE clock-gate stays at 2.4 GHz entering attention
                for w in range(24):
                    wp = psp.tile([128, 256], F32, tag="ps", bufs=3, name=f"wb{w}")
                    nc.tensor.matmul(wp, warm_l, warm_r[:, 0:256], start=True, stop=True)

            # ---- Stage B+C: attention, out-projection, ReduceScatter ----
            # Per 1024-wide l-superblock: heads 0/1 (PE row-groups 0/64) write
            # one [128,1024] S^T psum covering both -> ONE exp for the pair;
            # head 2 fills the two 512 halves of the same l-superblock via its
            # duplicated Q/K rows (also row-group paired, same mask bias).
            # O^T accumulates [V|1] over s-chunks (ones col = softmax denom).
            # Then normalize, duplicate rows, paired Z matmuls, ReduceScatter.
            zmm = [0]  # running parity for Z row-group pairing

            def norm_drain(h, oH, lsl, drp):
                lnm = f"{h}_{lsl.start}"
                dr = drp.tile([65, 512], F32, tag="dr", bufs=3, name=f"dr{lnm}")
                nc.vector.reciprocal(dr[64:65, :], oH[64:65])
                nc.sync.dma_start(out=dr[0:1, :], in_=dr[64:65, :])
                rb = drp.tile([64, 512], F32, tag="rb", bufs=3, name=f"rb{lnm}")
                nc.gpsimd.partition_broadcast(rb, dr[0:1, :])
                nc.vector.tensor_mul(OTn[h][0:64, lsl], oH[0:64], rb)
                nc.sync.dma_start(out=OTn[h][64:128, lsl], in_=OTn[h][0:64, lsl])

            with (
                tc.tile_pool(name="ptp", bufs=1) as ptp,
                tc.tile_pool(name="drp", bufs=1) as drp,
                tc.tile_pool(name="zsb", bufs=3) as zsb,
                tc.tile_pool(name="aps", bufs=1, space="PSUM") as aps,
            ):
                for sb2 in range(2):  # 1024-wide l superblocks
                    l0 = 1024 * sb2
                    # heads 0+1, two 512 l-blocks
                    for half in range(2):
                        lsl = slice(l0 + 512 * half, l0 + 512 * (half + 1))
                        oA = aps.tile([128, 512], F32, tag="oA", bufs=2, name="oA")
                        oB = aps.tile([128, 512], F32, tag="oB", bufs=2, name="oB")
                        for sc in range(SCH):
                            ssl = slice(128 * sc, 128 * (sc + 1))
                            sA = aps.tile([128, 512], F32, tag="sA", bufs=2, name="sA")
                            sB = aps.tile([128, 512], F32, tag="sB", bufs=2, name="sB")
                            nc.tensor.matmul(
                                sA, KT1[0:64, ssl], QT1[0:64, lsl], start=True, stop=True
                            )
                            nc.tensor.matmul(
                                sB,
                                KT1[64:128, ssl],
                                QT1[64:128, lsl],
                                start=True,
                                stop=True,
                            )
                            pA = ptp.tile([128, 512], F32R, tag="pA", bufs=3, name="pA")
                            pB = ptp.tile([128, 512], F32R, tag="pB", bufs=3, name="pB")
                            nc.scalar.activation(
                                pA, sA, ACT.Exp, bias=mask_bias[:, sc : sc + 1], scale=0.125
                            )
                            nc.scalar.activation(
                                pB, sB, ACT.Exp, bias=mask_bias[:, sc : sc + 1], scale=0.125
                            )
                            nc.tensor.matmul(
                                oA[0:65],
                                V_sb[:, sc, 0, :],
                                pA,
                                start=(sc == 0),
                                stop=(sc == SCH - 1),
                            )
                            nc.tensor.matmul(
                                oB[0:65],
                                V_sb[:, sc, 1, :],
                                pB,
                                start=(sc == 0),
                                stop=(sc == SCH - 1),
                            )
                        norm_drain(0, oA, lsl, drp)
                        norm_drain(1, oB, lsl, drp)
                    # head 2: both 512 halves of the superblock in one pass
                    lslA = slice(l0, l0 + 512)
                    lslB = slice(l0 + 512, l0 + 1024)
                    oA = aps.tile([128, 512], F32, tag="oA", bufs=2, name="oC1")
                    oB = aps.tile([128, 512], F32, tag="oB", bufs=2, name="oC2")
                    for sc in range(SCH):
                        ssl = slice(128 * sc, 128 * (sc + 1))
                        sA = aps.tile([128, 512], F32, tag="sA", bufs=2, name="sC1")
                        sB = aps.tile([128, 512], F32, tag="sB", bufs=2, name="sC2")
                        nc.tensor.matmul(
                            sA, KT2[0:64, ssl], QT2[0:64, lslA], start=True, stop=True
                        )
                        nc.tensor.matmul(
                            sB, KT2[64:128, ssl], QT2[64:128, lslB], start=True, stop=True
                        )
                        pA = ptp.tile([128, 512], F32R, tag="pA", bufs=3, name="pC1")
                        pB = ptp.tile([128, 512], F32R, tag="pB", bufs=3, name="pC2")
                        nc.scalar.activation(
                            pA, sA, ACT.Exp, bias=mask_bias[:, sc : sc + 1], scale=0.125
                        )
                        nc.scalar.activation(
                            pB, sB, ACT.Exp, bias=mask_bias[:, sc : sc + 1], scale=0.125
                        )
                        nc.tensor.matmul(
                            oA[0:65],
                            V_sb[:, sc, 2, :],
                            pA,
                            start=(sc == 0),
                            stop=(sc == SCH - 1),
                        )
                        nc.tensor.matmul(
                            oB[0:65],
                            V_sb[:, sc, 2, :],
                            pB,
                            start=(sc == 0),
                            stop=(sc == SCH - 1),
                        )
                    norm_drain(2, oA, lslA, drp)
                    norm_drain(2, oB, lslB, drp)

                    # out-projection for this superblock (8 l-tiles), Z matmuls
                    # row-group paired via the duplicated OTn/wTh rows
                    for lt in range(8 * sb2, 8 * (sb2 + 1)):
                        tsl = slice(128 * lt, 128 * (lt + 1))
                        zp1 = aps.tile([128, 512], F32, tag="oA", bufs=2, name="zp1")
                        zp2 = aps.tile([128, 256], F32, tag="oB", bufs=2, name="zp2")
                        for n0, nw, zp in ((0, 512, zp1), (512, 256, zp2)):
                            nsl = slice(n0, n0 + nw)
                            for h in range(HPC):
                                nc.tensor.matmul(
                                    zp[:, 0:nw],
                                    OTn[h][0:64, tsl],
                                    wTh_sb[h][0:64, nsl],
                                    start=(h == 0),
                                    stop=(h == HPC - 1),
                                )
                        zs = zsb.tile([128, D], F32, name="zs")
                        nc.vector.tensor_copy(out=zs[:, 0:512], in_=zp1)
                        nc.vector.tensor_copy(out=zs[:, 512:768], in_=zp2)
                        nc.sync.dma_start(out=Z_dram[tsl, :], in_=zs)
                        if lt % 4 == 3:
                            j = lt // 4
                            nc.gpsimd.collective_compute(
                                "ReduceScatter",
                                AL.add,
                                replica_groups=GROUPS,
                                ins=[Z_dram[512 * j : 512 * (j + 1), :].opt()],
                                outs=[Zr_dram[j].opt()],
                            )

            # ---- Stage D: bias + residual + LayerNorm ----
            with tc.tile_pool(name="ep", bufs=2) as ep:
                for t in range(LBN):
                    zr = ep.tile([128, D], F32, name="zr")
                    qr = ep.tile([128, D], F32, name="qr")
                    nc.sync.dma_start(out=zr, in_=Zr_dram[t])
                    nc.sync.dma_start(out=qr, in_=qrows[t])
                    x = ep.tile([128, D], F32, name="x")
                    nc.vector.tensor_add(x, zr, qr)
                    nc.vector.tensor_add(x, x, bb_b)
                    stats = ep.tile([128, 3, 6], F32, name="stats")
                    for g in range(3):
                        nc.vector.bn_stats(stats[:, g, :], x[:, 256 * g : 256 * (g + 1)])
                    mv = ep.tile([128, 2], F32, name="mv")
                    nc.vector.bn_aggr(mv, stats)
                    rstd = ep.tile([128, 1], F32, name="rstd")
                    nc.scalar.activation(rstd, mv[:, 1:2], ACT.Sqrt, bias=eps_t, scale=1.0)
                    nc.vector.reciprocal(rstd, rstd)
                    t1 = ep.tile([128, D], F32, name="t1")
                    nc.vector.scalar_tensor_tensor(
                        t1, x, mv[:, 0:1], gam_b, AL.subtract, AL.mult
                    )
                    o = ep.tile([128, D], F32, name="o")
                    nc.vector.scalar_tensor_tensor(
                        o, t1, rstd, bet_b, AL.mult, AL.add
                    )
                    nc.sync.dma_start(out=out[t], in_=o)

    nc.finalize()
    return nc


def _get_nc():
    if "nc" not in _CACHE:
        _CACHE["nc"] = _build()
    return _CACHE["nc"]


def build_in_maps(inputs):
    return _build_in_maps(**inputs)


def _build_in_maps(q, k, v, attention_mask, Wq, Wk, Wv, W, b, gamma, beta):
    q = np.asarray(q, dtype=np.float32)
    k = np.asarray(k, dtype=np.float32)
    v = np.asarray(v, dtype=np.float32)
    attention_mask = np.asarray(attention_mask, dtype=np.int32)
    Wq = np.asarray(Wq, dtype=np.float32)
    Wk = np.asarray(Wk, dtype=np.float32)
    Wv = np.asarray(Wv, dtype=np.float32)
    W = np.asarray(W, dtype=np.float32)
    b = np.asarray(b, dtype=np.float32)
    gamma = np.asarray(gamma, dtype=np.float32)
    beta = np.asarray(beta, dtype=np.float32)

    qT = [np.ascontiguousarray(q[i].T) for i in range(B)]
    kT = [np.ascontiguousarray(k[i].T) for i in range(B)]
    vT = [np.ascontiguousarray(v[i].T) for i in range(B)]
    maskT = [np.ascontiguousarray(attention_mask[i].reshape(SCH, 128).T) for i in range(B)]
    bias1 = np.ascontiguousarray(b.reshape(1, D))
    gamma1 = np.ascontiguousarray(gamma.reshape(1, D))
    beta1 = np.ascontiguousarray(beta.reshape(1, D))

    in_maps = []
    for c in range(NCORES):
        bi, hg = c // 4, c % 4
        cs = slice(HCOLS * hg, HCOLS * (hg + 1))
        wvT_pad = np.zeros((D, 256), dtype=np.float32)
        wvT_pad[:, :HCOLS] = Wv[cs, :].T
        wT = np.ascontiguousarray(W[:, cs].T)
        in_maps.append(
            {
                "qT": qT[bi],
                "kT": kT[bi],
                "vT": vT[bi],
                "wqT": np.ascontiguousarray(Wq[cs, :].T),
                "wkT": np.ascontiguousarray(Wk[cs, :].T),
                "wvT": wvT_pad,
                "wTh0": np.ascontiguousarray(np.concatenate([wT[0:64], wT[0:64]])),
                "wTh1": np.ascontiguousarray(np.concatenate([wT[64:128], wT[64:128]])),
                "wTh2": np.ascontiguousarray(np.concatenate([wT[128:192], wT[128:192]])),
                "qrows": np.ascontiguousarray(
                    np.stack(
                        [
                            q[bi, 512 * j + 128 * hg : 512 * j + 128 * (hg + 1), :]
                            for j in range(LBN)
                        ]
                    )
                ),
                "maskT": maskT[bi],
                "bias1": bias1,
                "gamma1": gamma1,
                "beta1": beta1,
            }
        )
    return in_maps


def kernel(q, k, v, attention_mask, Wq, Wk, Wv, W, b, gamma, beta):
    nc = _get_nc()
    in_maps = _build_in_maps(q, k, v, attention_mask, Wq, Wk, Wv, W, b, gamma, beta)
    res = run_bass_kernel_spmd(nc, in_maps, core_ids=list(range(NCORES)))

    outp = np.empty((B, L, D), dtype=np.float32)
    for c in range(NCORES):
        bi, hg = c // 4, c % 4
        o = res.results[c]["out"]
        for j in range(LBN):
            outp[bi, 512 * j + 128 * hg : 512 * j + 128 * (hg + 1), :] = o[j]
    return outp



# revision 9
# speedup vs baseline: 1.4196x; 1.4196x over previous
"""Trainium2 Bass kernel for fused MultiHeadAttention + residual + LayerNorm.

Problem: B=2, L=S=2048, D=768, H=12 heads of dim 64, attention with key-padding
mask, output projection + bias, residual with q, LayerNorm(gamma, beta).

Sharding over 8 NeuronCores: data-parallel over batch (2 groups of 4 cores) x
tensor-parallel over heads (3 heads per core).

V2: all matmul operands in bf16 (f32 PSUM accumulation), l-block-sequential
attention with row-group-paired score matmuls, exp split between ScalarE
(true exp, wide [128,1024] activations) and DVE (one-op Schraudolph fast-exp
written as int16 and bitcast to bf16), softmax denominator via an extra ones
column on V, normalization via reciprocal_approx_fast + gpsimd broadcast,
K=128 head-stacked output projection, bf16 ReduceScatter in 8 chunks each
followed immediately by its LayerNorm (no ScalarE table switches: rstd via a
DVE Newton rsqrt).
"""

import sys

sys.path.insert(0, "/opt/trn_rl_repo")

import ml_dtypes
import numpy as np

import concourse.bass as bass
import concourse.tile as tile
from concourse import bacc, mybir
from concourse.bass_utils import run_bass_kernel_spmd

F32 = mybir.dt.float32
BF16 = mybir.dt.bfloat16
I16 = mybir.dt.int16
I32 = mybir.dt.int32

D = 768
HD = 64
HPC = 3  # heads per core
HCOLS = HPC * HD  # 192
B, L, S = 2, 2048, 2048
NCORES = 8
GROUPS = [[0, 1, 2, 3], [4, 5, 6, 7]]
KCH = D // 128  # 6 contraction chunks for projections
SCH = S // 128  # 16 s-chunks
LB = 512  # l-block width
NLB = L // LB  # 4
NCHUNK = 8  # ReduceScatter chunks (256 rows each)
CROWS = L // NCHUNK  # 256
ORows = CROWS // 4  # 64 rows per core per chunk
LN_EPS = 1e-5
MASK_NEG = -1000000.0

# Schraudolph fast-exp into bf16 bits: bits = (s + b_p) * FE_A with
# FE_A = 0.125 * 128/ln2 and b_p = 8*mask_bias + FE_B/FE_A.
FE_A = 0.125 * (128.0 / np.log(2.0))  # 23.0831...
FE_B = 127.0 * 128.0 - 7.42 + 0.5  # bias - Schraudolph C + trunc hedge
FE_B_OVER_A = FE_B / FE_A

# exp engine split knobs
WIDE_SCALAR = (True, True, False, True, True, False, True, False)  # 5/3 per 8
NARROW_SCALAR = (True, False, True, False, True, False, True, False)

_CACHE: dict = {}


def _build():
    nc = bacc.Bacc("TRN2", target_bir_lowering=False, debug=False, num_devices=NCORES)

    qT = nc.dram_tensor("qT", [D, L], BF16, kind="ExternalInput").ap()
    kT = nc.dram_tensor("kT", [D, S], BF16, kind="ExternalInput").ap()
    vT = nc.dram_tensor("vT", [D, S], BF16, kind="ExternalInput").ap()
    wqT = nc.dram_tensor("wqT", [D, HCOLS], BF16, kind="ExternalInput").ap()
    wkT = nc.dram_tensor("wkT", [D, HCOLS], BF16, kind="ExternalInput").ap()
    wvT = nc.dram_tensor("wvT", [D, HCOLS], BF16, kind="ExternalInput").ap()
    wT01 = nc.dram_tensor("wT01", [128, D], BF16, kind="ExternalInput").ap()
    wT2 = nc.dram_tensor("wT2", [64, D], BF16, kind="ExternalInput").ap()
    qres = nc.dram_tensor("qres", [NCHUNK, ORows, D], BF16, kind="ExternalInput").ap()
    maskT = nc.dram_tensor("maskT", [128, SCH], I32, kind="ExternalInput").ap()
    bias1 = nc.dram_tensor("bias1", [1, D], F32, kind="ExternalInput").ap()
    gamma1 = nc.dram_tensor("gamma1", [1, D], F32, kind="ExternalInput").ap()
    beta1 = nc.dram_tensor("beta1", [1, D], F32, kind="ExternalInput").ap()
    out = nc.dram_tensor("out", [NCHUNK, ORows, D], F32, kind="ExternalOutput").ap()

    AL = mybir.AluOpType
    ACT = mybir.ActivationFunctionType

    with tile.TileContext(nc, num_cores=NCORES) as tc:
        with (
            tc.tile_pool(name="persist", bufs=1) as pp,
            tc.tile_pool(name="dram", bufs=1, space="DRAM") as dram,
        ):
            QT1 = pp.tile([128, L], BF16)  # h0 feats on 0:64, h1 on 64:128
            QT2 = pp.tile([128, L], BF16)  # h2 feats, duplicated 64:128
            KT1 = pp.tile([128, S], BF16)
            KT2 = pp.tile([128, S], BF16)
            V_sb = pp.tile([128, SCH, HPC, 65], BF16)  # col 64 = ones (denom)
            OT01 = pp.tile([128, L], BF16)  # normalized O^T: h0 top, h1 bottom
            OT2 = pp.tile([64, L], BF16)
            wq_sb = pp.tile([128, KCH, HCOLS], BF16)
            wk_sb = pp.tile([128, KCH, HCOLS], BF16)
            wv_sb = pp.tile([128, KCH, HCOLS], BF16)
            wT01_sb = pp.tile([128, D], BF16)
            wT2_sb = pp.tile([64, D], BF16)
            mask_i = pp.tile([128, SCH], I32)
            mask_f = pp.tile([128, SCH], F32)
            asc_bias = pp.tile([128, SCH], F32)  # ScalarE exp bias
            dve_bias = pp.tile([128, SCH], F32)  # DVE fast-exp bias
            gam_b = pp.tile([128, D], F32)
            bet_b = pp.tile([128, D], F32)
            bb_b = pp.tile([128, D], F32)

            Z_dram = dram.tile([L, D], BF16)
            Zr_dram = dram.tile([NCHUNK, ORows, D], BF16)

            # constant / weight loads
            nc.sync.dma_start(out=wq_sb, in_=wqT.rearrange("(c p) m -> p c m", p=128))
            nc.sync.dma_start(out=wk_sb, in_=wkT.rearrange("(c p) m -> p c m", p=128))
            nc.sync.dma_start(out=wv_sb, in_=wvT.rearrange("(c p) m -> p c m", p=128))
            nc.sync.dma_start(out=wT01_sb, in_=wT01[:, :])
            nc.sync.dma_start(out=wT2_sb, in_=wT2[:, :])
            nc.sync.dma_start(out=mask_i, in_=maskT[:, :])
            nc.sync.dma_start(out=gam_b, in_=gamma1.to_broadcast([128, D]))
            nc.sync.dma_start(out=bet_b, in_=beta1.to_broadcast([128, D]))
            nc.sync.dma_start(out=bb_b, in_=bias1.to_broadcast([128, D]))
            ones_t = pp.tile([128, SCH, HPC, 1], BF16)
            nc.vector.memset(ones_t, 1.0)
            nc.vector.tensor_copy(V_sb[:, :, :, 64:65], ones_t)
            nc.vector.tensor_copy(mask_f, mask_i)
            # ScalarE: bias = (1-m)*MASK_NEG == m*(-MASK_NEG) + MASK_NEG
            nc.scalar.activation(
                asc_bias, mask_f, ACT.Copy, bias=float(MASK_NEG), scale=-MASK_NEG
            )
            # DVE: b_p = 8*mask_bias + FE_B/FE_A = m*(-8*MASK_NEG) + 8*MASK_NEG + FE_B/FE_A
            nc.scalar.activation(
                dve_bias,
                mask_f,
                ACT.Copy,
                bias=float(8.0 * MASK_NEG + FE_B_OVER_A),
                scale=-8.0 * MASK_NEG,
            )

            # PE warm-up during the initial DMA window
            warm_f = pp.tile([128, 512], F32)
            nc.vector.memset(warm_f, 0.0)
            warm_l = pp.tile([128, 128], BF16)
            warm_r = pp.tile([128, 512], BF16)
            nc.vector.tensor_copy(warm_l, warm_f[:, 0:128])
            nc.vector.tensor_copy(warm_r, warm_f)
            with tc.tile_pool(name="warmps", bufs=1, space="PSUM") as wps:
                for w in range(40):
                    wp = wps.tile([128, 512], F32, tag="w", bufs=2, name=f"w{w}")
                    nc.tensor.matmul(wp, warm_l, warm_r, start=True, stop=True)

            # ---- Stage A: projections (all bf16) ----
            with (
                tc.tile_pool(name="pin", bufs=1) as pin,
                tc.tile_pool(name="psp", bufs=1, space="PSUM") as psp,
            ):
                kch = []
                vch = []
                qch = []
                for i in range(KCH):
                    ch = pin.tile([128, S], BF16, tag="kin", bufs=KCH, name=f"kch{i}")
                    nc.sync.dma_start(out=ch, in_=kT[128 * i : 128 * (i + 1), :])
                    kch.append(ch)
                for i in range(KCH):
                    ch = pin.tile([128, S], BF16, tag="vin", bufs=KCH, name=f"vch{i}")
                    nc.sync.dma_start(out=ch, in_=vT[128 * i : 128 * (i + 1), :])
                    vch.append(ch)
                for i in range(KCH):
                    ch = pin.tile([128, L], BF16, tag="qin", bufs=KCH, name=f"qch{i}")
                    nc.sync.dma_start(out=ch, in_=qT[128 * i : 128 * (i + 1), :])
                    qch.append(ch)

                # K heads 0/1 -> KT1 (full 128 feature rows)
                for n in range(NLB):
                    nsl = slice(512 * n, 512 * (n + 1))
                    ps = psp.tile([128, 512], F32, tag="pA", bufs=3, name="psk")
                    for i in range(KCH):
                        nc.tensor.matmul(
                            ps,
                            wk_sb[:, i, 0:128],
                            kch[i][:, nsl],
                            start=(i == 0),
                            stop=(i == KCH - 1),
                        )
                    nc.vector.tensor_copy(out=KT1[:, nsl], in_=ps)

                # V projection -> V_sb[:, s, h, 0:64]
                for s in range(SCH):
                    ps = psp.tile([128, 192], F32, tag="pB", bufs=3, name="psv")
                    for i in range(KCH):
                        nc.tensor.matmul(
                            ps,
                            vch[i][:, 128 * s : 128 * (s + 1)],
                            wv_sb[:, i, :],
                            start=(i == 0),
                            stop=(i == KCH - 1),
                        )
                    nc.scalar.copy(
                        out=V_sb[:, s, :, 0:64],
                        in_=ps.rearrange("p (h d) -> p h d", h=HPC),
                    )

                # Q heads 0/1 -> QT1
                for n in range(NLB):
                    nsl = slice(512 * n, 512 * (n + 1))
                    ps = psp.tile([128, 512], F32, tag="pA", bufs=3, name="psq")
                    for i in range(KCH):
                        nc.tensor.matmul(
                            ps,
                            wq_sb[:, i, 0:128],
                            qch[i][:, nsl],
                            start=(i == 0),
                            stop=(i == KCH - 1),
                        )
                    nc.vector.tensor_copy(out=QT1[:, nsl], in_=ps)

                # head-2 Q and K projections, col-group paired (concurrent):
                # Q-m1 writes psum partitions 0:64 (col groups 0/1), K-m1 a
                # separate psum's partitions 64:128 (col groups 2/3).
                for n in range(NLB):
                    nsl = slice(512 * n, 512 * (n + 1))
                    psq2 = psp.tile([128, 512], F32, tag="pA", bufs=3, name="psq2")
                    psk2 = psp.tile([128, 512], F32, tag="pC", bufs=2, name="psk2")
                    for i in range(KCH):
                        nc.tensor.matmul(
                            psq2[0:64],
                            wq_sb[:, i, 128:192],
                            qch[i][:, nsl],
                            start=(i == 0),
                            stop=(i == KCH - 1),
                        )
                        nc.tensor.matmul(
                            psk2[64:128],
                            wk_sb[:, i, 128:192],
                            kch[i][:, nsl],
                            start=(i == 0),
                            stop=(i == KCH - 1),
                        )
                    nc.vector.tensor_copy(out=QT2[0:64, nsl], in_=psq2[0:64])
                    nc.vector.tensor_copy(out=KT2[0:64, nsl], in_=psk2[64:128])
                    nc.sync.dma_start(out=QT2[64:128, nsl], in_=QT2[0:64, nsl])
                    nc.sync.dma_start(out=KT2[64:128, nsl], in_=KT2[0:64, nsl])

                # keep-warm bridge across the pool transition
                for w in range(16):
                    wp = psp.tile([128, 512], F32, tag="pB", bufs=3, name=f"wb{w}")
                    nc.tensor.matmul(wp, warm_l, warm_r, start=True, stop=True)

            # ---- Stage B: attention + out-projection + RS + LN, per l-block ----
            def fexp_dve(dst_bf16, src_psum, scslice):
                # bf16 bits = (s + b_p) * FE_A, written as int16 (bitcast view)
                nc.vector.tensor_scalar(
                    out=dst_bf16.bitcast(I16),
                    in0=src_psum,
                    scalar1=scslice,
                    scalar2=float(FE_A),
                    op0=AL.add,
                    op1=AL.mult,
                )

            def norm_drain(o_ps, dest, lnm, drp):
                # dest <- o_ps[0:64] * (1 / denom_row) ; denom = o_ps[64:65]
                dr = drp.tile([1, 512], F32, tag="dr", bufs=2, name=f"dr{lnm}")
                nc.scalar.copy(out=dr, in_=o_ps[64:65, :])
                rr = drp.tile([1, 512], F32, tag="rr", bufs=2, name=f"rr{lnm}")
                nc.vector.reciprocal_approx_fast(rr, dr)
                rb = drp.tile([64, 512], F32, tag="rb", bufs=2, name=f"rb{lnm}")
                nc.gpsimd.partition_broadcast(rb, rr)
                nc.vector.tensor_mul(dest, o_ps[0:64, :], rb)

            with (
                tc.tile_pool(name="ptp", bufs=1) as ptp,
                tc.tile_pool(name="drp", bufs=1) as drp,
                tc.tile_pool(name="zsb", bufs=3) as zsb,
                tc.tile_pool(name="aps", bufs=1, space="PSUM") as aps,
                tc.tile_pool(name="ep", bufs=2) as ep,
            ):
                for lb in range(NLB):
                    lsl = slice(512 * lb, 512 * (lb + 1))
                    # heads 0+1: row-group paired scores, wide exp, PV (M=65)
                    oA = aps.tile([65, 512], F32, tag="oA", bufs=1, name=f"oA{lb}")
                    oB = aps.tile([65, 512], F32, tag="oB", bufs=1, name=f"oB{lb}")
                    for sc in range(SCH):
                        ssl = slice(128 * sc, 128 * (sc + 1))
                        sw = aps.tile(
                            [128, 1024], F32, tag="sw", bufs=2, name=f"sw{lb}_{sc}"
                        )
                        nc.tensor.matmul(
                            sw[:, 0:512], KT1[0:64, ssl], QT1[0:64, lsl],
                            start=True, stop=True,
                        )
                        nc.tensor.matmul(
                            sw[:, 512:1024], KT1[64:128, ssl], QT1[64:128, lsl],
                            start=True, stop=True,
                        )
                        P = ptp.tile([128, 1024], BF16, tag="p", bufs=3, name="P")
                        if WIDE_SCALAR[sc % 8]:
                            nc.scalar.activation(
                                P, sw, ACT.Exp,
                                bias=asc_bias[:, sc : sc + 1], scale=0.125,
                            )
                        else:
                            fexp_dve(P[:, 0:1024], sw[:, 0:1024], dve_bias[:, sc : sc + 1])
                        nc.tensor.matmul(
                            oA, V_sb[:, sc, 0, :], P[:, 0:512],
                            start=(sc == 0), stop=(sc == SCH - 1),
                        )
                        nc.tensor.matmul(
                            oB, V_sb[:, sc, 1, :], P[:, 512:1024],
                            start=(sc == 0), stop=(sc == SCH - 1),
                        )
                    norm_drain(oA, OT01[0:64, lsl], f"a{lb}", drp)
                    norm_drain(oB, OT01[64:128, lsl], f"b{lb}", drp)

                    # head 2: paired across consecutive s-chunks
                    oC = aps.tile([65, 512], F32, tag="oC", bufs=1, name=f"oC{lb}")
                    for sd in range(SCH // 2):
                        sc0, sc1 = 2 * sd, 2 * sd + 1
                        ssl0 = slice(128 * sc0, 128 * (sc0 + 1))
                        ssl1 = slice(128 * sc1, 128 * (sc1 + 1))
                        sw2 = aps.tile(
                            [128, 1024], F32, tag="sw", bufs=2, name=f"sw2_{lb}_{sd}"
                        )
                        nc.tensor.matmul(
                            sw2[:, 0:512], KT2[0:64, ssl0], QT2[0:64, lsl],
                            start=True, stop=True,
                        )
                        nc.tensor.matmul(
                            sw2[:, 512:1024], KT2[64:128, ssl1], QT2[64:128, lsl],
                            start=True, stop=True,
                        )
                        P2 = ptp.tile([128, 1024], BF16, tag="p", bufs=3, name="P2")
                        for half, scx in ((0, sc0), (1, sc1)):
                            hs = slice(512 * half, 512 * (half + 1))
                            if NARROW_SCALAR[(2 * sd + half) % 8]:
                                nc.scalar.activation(
                                    P2[:, hs], sw2[:, hs], ACT.Exp,
                                    bias=asc_bias[:, scx : scx + 1], scale=0.125,
                                )
                            else:
                                fexp_dve(
                                    P2[:, hs], sw2[:, hs], dve_bias[:, scx : scx + 1]
                                )
                        nc.tensor.matmul(
                            oC, V_sb[:, sc0, 2, :], P2[:, 0:512],
                            start=(sd == 0), stop=False,
                        )
                        nc.tensor.matmul(
                            oC, V_sb[:, sc1, 2, :], P2[:, 512:1024],
                            start=False, stop=(sd == SCH // 2 - 1),
                        )
                    norm_drain(oC, OT2[:, lsl], f"c{lb}", drp)

                    # out-projection for this l-block: Z = [O0;O1]^T.W01 + O2^T.W2
                    for t in range(4):
                        lt = 4 * lb + t
                        tsl = slice(128 * lt, 128 * (lt + 1))
                        zp = aps.tile(
                            [128, 1024], F32, tag="sw", bufs=2, name=f"zp{lt}"
                        )
                        for n0, nw in ((0, 512), (512, 256)):
                            nsl = slice(n0, n0 + nw)
                            zsl = slice(n0, n0 + nw)
                            nc.tensor.matmul(
                                zp[:, zsl], OT01[:, tsl], wT01_sb[:, nsl],
                                start=True, stop=False,
                            )
                            nc.tensor.matmul(
                                zp[:, zsl], OT2[:, tsl], wT2_sb[:, nsl],
                                start=False, stop=True,
                            )
                        zs = zsb.tile([128, D], BF16, name="zs")
                        nc.scalar.copy(out=zs, in_=zp[:, 0:768])
                        nc.sync.dma_start(out=Z_dram[tsl, :], in_=zs)

                        if t % 2 == 1:
                            k = lt // 2
                            nc.gpsimd.collective_compute(
                                "ReduceScatter",
                                AL.add,
                                replica_groups=GROUPS,
                                ins=[Z_dram[CROWS * k : CROWS * (k + 1), :].opt()],
                                outs=[Zr_dram[k].opt()],
                            )
                            # ---- LayerNorm for chunk k (rows: ORows) ----
                            zr = ep.tile([ORows, D], BF16, name="zr")
                            qr = ep.tile([ORows, D], BF16, name="qr")
                            nc.sync.dma_start(out=zr, in_=Zr_dram[k])
                            nc.sync.dma_start(out=qr, in_=qres[k])
                            xb = ep.tile([ORows, D], F32, name="xb")
                            nc.vector.tensor_add(xb, zr, qr)
                            stats = ep.tile([ORows, 3, 6], F32, name="stats")
                            for g in range(3):
                                nc.vector.bn_stats(
                                    stats[:, g, :], xb[:, 256 * g : 256 * (g + 1)]
                                )
                            mv = ep.tile([ORows, 2], F32, name="mv")
                            nc.vector.bn_aggr(mv, stats)
                            # rstd = rsqrt(var + eps): bit-trick seed + 1 Newton
                            ve = ep.tile([ORows, 1], F32, name="ve")
                            nc.vector.tensor_scalar_add(ve, mv[:, 1:2], float(LN_EPS))
                            sh = ep.tile([ORows, 1], I32, name="sh")
                            nc.vector.tensor_scalar(
                                out=sh, in0=ve[:, 0:1].bitcast(I32), scalar1=1,
                                scalar2=None, op0=AL.arith_shift_right,
                            )
                            r0i = ep.tile([ORows, 1], I32, name="r0i")
                            nc.vector.tensor_scalar(
                                out=r0i, in0=sh, scalar1=0x5F3759DF, scalar2=-1,
                                op0=AL.subtract, op1=AL.mult,
                            )
                            r0 = r0i[:, 0:1].bitcast(F32)
                            t2 = ep.tile([ORows, 1], F32, name="t2")
                            nc.vector.tensor_mul(t2, r0, r0)
                            nc.vector.tensor_mul(t2, t2, ve)
                            nc.vector.tensor_scalar(
                                out=t2, in0=t2, scalar1=-0.5, scalar2=1.5,
                                op0=AL.mult, op1=AL.add,
                            )
                            rstd = ep.tile([ORows, 1], F32, name="rstd")
                            nc.vector.tensor_mul(rstd, r0, t2)
                            t1 = ep.tile([ORows, D], F32, name="t1")
                            nc.vector.scalar_tensor_tensor(
                                t1, xb, mv[:, 0:1], gam_b[0:ORows],
                                AL.subtract, AL.mult,
                            )
                            o = ep.tile([ORows, D], F32, name="o")
                            nc.vector.scalar_tensor_tensor(
                                o, t1, rstd, bet_b[0:ORows], AL.mult, AL.add
                            )
                            nc.sync.dma_start(out=out[k], in_=o)

    nc.finalize()
    return nc


def _get_nc():
    if "nc" not in _CACHE:
        _CACHE["nc"] = _build()
    return _CACHE["nc"]


def build_in_maps(inputs):
    return _build_in_maps(**inputs)


def _bf(x):
    return np.ascontiguousarray(np.asarray(x, dtype=np.float32).astype(ml_dtypes.bfloat16))


def _build_in_maps(q, k, v, attention_mask, Wq, Wk, Wv, W, b, gamma, beta):
    q = np.asarray(q, dtype=np.float32)
    k = np.asarray(k, dtype=np.float32)
    v = np.asarray(v, dtype=np.float32)
    attention_mask = np.asarray(attention_mask, dtype=np.int32)
    Wq = np.asarray(Wq, dtype=np.float32)
    Wk = np.asarray(Wk, dtype=np.float32)
    Wv = np.asarray(Wv, dtype=np.float32)
    W = np.asarray(W, dtype=np.float32)
    b = np.asarray(b, dtype=np.float32)
    gamma = np.asarray(gamma, dtype=np.float32)
    beta = np.asarray(beta, dtype=np.float32)

    qT = [_bf(q[i].T) for i in range(B)]
    kT = [_bf(k[i].T) for i in range(B)]
    vT = [_bf(v[i].T) for i in range(B)]
    maskT = [
        np.ascontiguousarray(attention_mask[i].reshape(SCH, 128).T) for i in range(B)
    ]
    bias1 = np.ascontiguousarray(b.reshape(1, D))
    gamma1 = np.ascontiguousarray(gamma.reshape(1, D))
    beta1 = np.ascontiguousarray(beta.reshape(1, D))

    in_maps = []
    for c in range(NCORES):
        bi, hg = c // 4, c % 4
        cs = slice(HCOLS * hg, HCOLS * (hg + 1))
        wT = W[:, cs].T  # [192, 768]
        in_maps.append(
            {
                "qT": qT[bi],
                "kT": kT[bi],
                "vT": vT[bi],
                "wqT": _bf(Wq[cs, :].T),
                "wkT": _bf(Wk[cs, :].T),
                "wvT": _bf(Wv[cs, :].T),
                "wT01": _bf(wT[0:128]),
                "wT2": _bf(wT[128:192]),
                "qres": _bf(
                    np.stack(
                        [
                            q[
                                bi,
                                CROWS * j + ORows * hg : CROWS * j + ORows * (hg + 1),
                                :,
                            ]
                            + b[None, :]
                            for j in range(NCHUNK)
                        ]
                    )
                ),
                "maskT": maskT[bi],
                "bias1": bias1,
                "gamma1": gamma1,
                "beta1": beta1,
            }
        )
    return in_maps


def kernel(q, k, v, attention_mask, Wq, Wk, Wv, W, b, gamma, beta):
    nc = _get_nc()
    in_maps = _build_in_maps(q, k, v, attention_mask, Wq, Wk, Wv, W, b, gamma, beta)
    res = run_bass_kernel_spmd(nc, in_maps, core_ids=list(range(NCORES)))

    outp = np.empty((B, L, D), dtype=np.float32)
    for c in range(NCORES):
        bi, hg = c // 4, c % 4
        o = res.results[c]["out"]
        for j in range(NCHUNK):
            outp[bi, CROWS * j + ORows * hg : CROWS * j + ORows * (hg + 1), :] = o[j]
    return outp
